# revision 1
# baseline (speedup 1.0000x reference)
"""Gemma3 sliding-window attention, tensor-parallel over heads on 8 trn2 cores.

Sharding: core d owns q heads [2d, 2d+1] and KV head d (GQA group stays
local), plus the matching row-slices of wq/wk/wv and column-slice of wo.
Each core computes its partial output projection; partials are summed on
the host (all-reduce equivalent).
"""
import numpy as np
import jax
import jax.numpy as jnp

B, T, HID = 2, 2048, 3840
H, KV, D = 16, 8, 256
EPS = 1e-6
NCORES = 8
HPC = H // NCORES      # q heads per core = 2
KVPC = KV // NCORES    # kv heads per core = 1
REP = HPC // KVPC


def _attn_shard(x, cos, sin, mask, wq_s, wk_s, wv_s, wo_s, qn, kn):
    # x: (B,T,HID)  wq_s: (HPC*D,HID)  wk_s/wv_s: (KVPC*D,HID)  wo_s: (HID,HPC*D)
    q = (x @ wq_s.T).reshape(B, T, HPC, D).transpose(0, 2, 1, 3)
    k = (x @ wk_s.T).reshape(B, T, KVPC, D).transpose(0, 2, 1, 3)
    v = (x @ wv_s.T).reshape(B, T, KVPC, D).transpose(0, 2, 1, 3)
    q = q * jax.lax.rsqrt(jnp.mean(jnp.square(q), axis=-1, keepdims=True) + EPS) * qn
    k = k * jax.lax.rsqrt(jnp.mean(jnp.square(k), axis=-1, keepdims=True) + EPS) * kn
    c = cos[None, None]
    s = sin[None, None]
    q1, q2 = q[..., : D // 2], q[..., D // 2 :]
    q = jnp.concatenate([q1 * c - q2 * s, q2 * c + q1 * s], axis=-1)
    k1, k2 = k[..., : D // 2], k[..., D // 2 :]
    k = jnp.concatenate([k1 * c - k2 * s, k2 * c + k1 * s], axis=-1)
    k = jnp.repeat(k, REP, axis=1)  # (B,HPC,T,D)
    v = jnp.repeat(v, REP, axis=1)
    scale = 1.0 / jnp.sqrt(jnp.float32(D))
    scores = jnp.einsum("bhqd,bhkd->bhqk", q, k) * scale + mask
    attn = jax.nn.softmax(scores, axis=-1)
    out = jnp.einsum("bhqk,bhkd->bhqd", attn, v)
    out = out.transpose(0, 2, 1, 3).reshape(B, T, HPC * D)
    return out @ wo_s.T  # (B,T,HID) partial


_pmapped = jax.pmap(_attn_shard)


def kernel(**inputs):
    x = np.asarray(inputs["x"], dtype=np.float32)
    cos = np.asarray(inputs["cos_local"], dtype=np.float32)  # layer 0 -> local rope
    sin = np.asarray(inputs["sin_local"], dtype=np.float32)
    mask = np.asarray(inputs["attention_mask"], dtype=np.float32)[0]  # (1,T,T)
    wq = np.asarray(inputs["wq"], dtype=np.float32)
    wk = np.asarray(inputs["wk"], dtype=np.float32)
    wv = np.asarray(inputs["wv"], dtype=np.float32)
    wo = np.asarray(inputs["wo"], dtype=np.float32)
    qn = np.asarray(inputs["q_norm_w"], dtype=np.float32)
    kn = np.asarray(inputs["k_norm_w"], dtype=np.float32)

    rep = lambda a: np.broadcast_to(a, (NCORES,) + a.shape)
    wq_s = wq.reshape(NCORES, HPC * D, HID)
    wk_s = wk.reshape(NCORES, KVPC * D, HID)
    wv_s = wv.reshape(NCORES, KVPC * D, HID)
    wo_s = np.ascontiguousarray(
        wo.reshape(HID, NCORES, HPC * D).transpose(1, 0, 2)
    )  # (NCORES, HID, HPC*D)

    partials = _pmapped(
        rep(x), rep(cos), rep(sin), rep(mask),
        wq_s, wk_s, wv_s, wo_s, rep(qn), rep(kn),
    )
    out = np.asarray(partials).sum(axis=0, dtype=np.float32)
    return out.astype(np.float32)
